# revision 1
# baseline (speedup 1.0000x reference)
"""Distributed Bass kernel for the DPhysics problem (8 TRN2 NeuronCores).

Layout: 8 robots/core; compute tiles [128, 64]: partition 16r+j, free f;
point i of robot r at (16r + i%16, i//16).
Terrain: GPSIMD ap_gather (d=2 pair windows, 128x128 cells, +-6.4 m).
Window partition 16r + 2g+s holds pairs (value, x-diff) of grid g, y-shift s.
Swap to compute layout: 128 masked stream_shuffles (DVE, mask=255 suppress).
Reductions/broadcasts: PE matmuls with one-hot block matrices.
"""

import numpy as np
import concourse.bass as bass
import concourse.bacc as bacc
import concourse.mybir as mybir
from concourse.tile import TileContext
from bass_rust import add_dep_helper

F32 = mybir.dt.float32
I32 = mybir.dt.int32
I16 = mybir.dt.int16
AF = mybir.ActivationFunctionType
OP = mybir.AluOpType

D_MAX = 12.8
RES = 0.1
DT = 0.01
MASS = 40.0
GRAV = 9.8
ROBOT_LY = 0.5
OMEGA_MAX = 7.0
T_STEPS = 64
NPTS = 1024
NF = 64
C_RK4 = DT * (1.0 + DT / 2.0 + DT * DT / 6.0 + DT ** 3 / 24.0)
WX0 = 96
WY0 = 96
WN = 64
NE = WN * WN
CLIP_LO = 96.0
CLIP_HI = 159.96875
BF16 = mybir.dt.bfloat16
W_SUB = 64 * 96 + 96  # offset subtracted from 64*xi+yi


def build(nsteps=T_STEPS, sim_safe_masks=False, debug=False,
          gps_eff=None):
    if gps_eff is not None:
        from concourse import hw_specs
        hw_specs.TRN2Spec.GPSIMD_IMPL_EFFICIENCY_DEFAULT = gps_eff
    nc = bacc.Bacc(target_bir_lowering=False)

    wins_d = nc.declare_dram_parameter("wins", [128, 8 * NE], BF16, isOutput=False)
    pts_d = nc.declare_dram_parameter("pts", [128, 3 * NF], F32, isOutput=False)
    msk_d = nc.declare_dram_parameter("msk", [128, 2 * NF], F32, isOutput=False)
    tv_d = nc.declare_dram_parameter("tv", [8, 2 * T_STEPS], F32, isOutput=False)
    l1_d = nc.declare_dram_parameter("l1", [128, 8], F32, isOutput=False)
    l2_d = nc.declare_dram_parameter("l2", [8, 128], F32, isOutput=False)
    sc_d = nc.declare_dram_parameter("sc", [8, 8], F32, isOutput=False)
    out_d = nc.declare_dram_parameter("out", [8, 3 * T_STEPS], F32, isOutput=True)
    if debug:
        dbg_d = nc.declare_dram_parameter("dbg", [128, 16 * NF], F32,
                                          isOutput=True)

    with TileContext(nc) as tc:
        with (
            tc.tile_pool(name="big", bufs=1) as big,
            tc.tile_pool(name="state", bufs=1) as state,
            tc.tile_pool(name="dstate", bufs=2) as dstate,
            tc.tile_pool(name="gath", bufs=2) as gath,
            tc.tile_pool(name="tmp", bufs=2) as tmp,
            tc.tile_pool(name="ps", bufs=2, space="PSUM") as pspool,
        ):
            WIN = big.tile([128, 8 * NE], BF16)
            MK = state.tile([128, 2 * NF], F32)
            TV = state.tile([8, 2 * T_STEPS], F32)
            L1 = state.tile([128, 8], F32)
            L2 = state.tile([8, 128], F32)
            SCT = state.tile([8, 8], F32)
            NEG = state.tile([128, NF], F32)
            RB = state.tile([8, 12], F32)
            XH = state.tile([8, 3 * T_STEPS], F32)
            PTS0 = state.tile([128, 3 * NF], F32)

            nc.sync.dma_start(out=WIN[:, :], in_=wins_d[:, :])
            nc.sync.dma_start(out=PTS0[:, :], in_=pts_d[:, :])
            nc.sync.dma_start(out=MK[:, :], in_=msk_d[:, :])
            nc.sync.dma_start(out=TV[:, :], in_=tv_d[:, :])
            nc.sync.dma_start(out=L1[:, :], in_=l1_d[:, :])
            nc.sync.dma_start(out=L2[:, :], in_=l2_d[:, :])
            nc.sync.dma_start(out=SCT[:, :], in_=sc_d[:, :])

            CB128 = state.tile([128, 1], F32)
            CPI2 = state.tile([128, 1], F32)
            Cm96 = state.tile([128, 1], F32)
            C6397 = state.tile([128, 1], F32)
            C15997 = state.tile([128, 1], F32)
            Cm6240 = state.tile([128, 1], F32)
            Cm05 = state.tile([128, 1], F32)
            nc.vector.memset(CB128[:, :], 128.0)
            nc.vector.memset(CPI2[:, :], float(np.pi / 2))
            nc.vector.memset(Cm96[:, :], -96.0)
            nc.vector.memset(C6397[:, :], 63.96875)
            nc.vector.memset(C15997[:, :], 159.96875)
            nc.vector.memset(Cm6240[:, :], -6240.0)
            nc.vector.memset(Cm05[:, :], -0.5)
            ONES = state.tile([128, NF], F32)
            nc.vector.memset(ONES[:, :], 1.0)
            nc.vector.memset(NEG[:, :], -1.0 / NPTS)
            nc.vector.memset(RB[:, 0:12], 0.0)
            nc.vector.memset(RB[:, 0:1], 1.0)
            nc.vector.memset(RB[:, 4:5], 1.0)
            nc.vector.memset(RB[:, 8:9], 1.0)

            def fresh(name, shape=(128, NF), dtype=F32):
                return tmp.tile(list(shape), dtype, name=name)

            def idx_block(Xc, Yc, tag):
                # xs = clip(10*X + 128, 96, 159.96875) via Relu chain (ACT)
                xs = fresh("xs" + tag)
                ys = fresh("ys" + tag)
                for (o, i_) in ((xs, Xc), (ys, Yc)):
                    nc.scalar.activation(o[:, :], i_, AF.Identity,
                                         bias=CB128[:, 0:1], scale=10.0)
                    nc.scalar.activation(o[:, :], o[:, :], AF.Relu,
                                         bias=Cm96[:, 0:1])
                    nc.scalar.activation(o[:, :], o[:, :], AF.Relu,
                                         bias=C6397[:, 0:1], scale=-1.0)
                    nc.scalar.activation(o[:, :], o[:, :], AF.Identity,
                                         bias=C15997[:, 0:1], scale=-1.0)
                # floor via round(x - 0.5) (exact-int x gives same interp)
                xsh = fresh("xsh" + tag)
                ysh = fresh("ysh" + tag)
                nc.scalar.activation(xsh[:, :], xs[:, :], AF.Identity,
                                     bias=Cm05[:, 0:1])
                nc.scalar.activation(ysh[:, :], ys[:, :], AF.Identity,
                                     bias=Cm05[:, 0:1])
                xi = fresh("xi" + tag, dtype=I32)
                yi = fresh("yi" + tag, dtype=I32)
                xif = fresh("xif" + tag)
                yif = fresh("yif" + tag)
                nc.scalar.copy(xi[:, :], xsh[:, :])
                nc.scalar.copy(yi[:, :], ysh[:, :])
                nc.scalar.copy(xif[:, :], xi[:, :])
                nc.scalar.copy(yif[:, :], yi[:, :])
                xf = fresh("xf" + tag)
                yf = fresh("yf" + tag)
                nc.vector.tensor_tensor(out=xf[:, :], in0=xs[:, :],
                                        in1=xif[:, :], op=OP.subtract)
                nc.vector.tensor_tensor(out=yf[:, :], in0=ys[:, :],
                                        in1=yif[:, :], op=OP.subtract)
                # w = 64*xif + yif - 6240
                wh = fresh("wh" + tag)
                nc.scalar.activation(wh[:, :], xif[:, :], AF.Identity,
                                     bias=Cm6240[:, 0:1], scale=64.0)
                wf = fresh("wf" + tag)
                nc.vector.tensor_tensor(out=wf[:, :], in0=wh[:, :],
                                        in1=yif[:, :], op=OP.add)
                idx = fresh("idx" + tag, dtype=I16)
                ii = nc.scalar.copy(idx[:, :], wf[:, :])
                return idx, xf, yf, ii

            def swap_half(G, TT, f0):
                # G: [128, 4096] bf16 = 512 idx x oct; writes TT f-cols
                # [f0, f0+32) of both variant blocks
                Gf = G[:, :].bitcast(F32)      # [128, 2048] f32 words
                Gv = Gf.rearrange("p (m e) -> p m e", e=4)
                TTf = TT[:, :].bitcast(F32)
                for k in range(2):
                    dst = TTf[:, 256 * k:256 * (k + 1)].rearrange(
                        "p (f e) -> p f e", e=4)[:, f0:f0 + 32, :]
                    for v in range(16):
                        if sim_safe_masks:
                            mask = [16 * (u // 16) + k for u in range(32)]
                        else:
                            mask = [255] * 32
                            mask[v] = k
                            mask[16 + v] = 16 + k
                        nc.vector.stream_shuffle(
                            out=dst, in_=Gv[:, v:512:16, :], mask=mask)

            def swap_block(GA, GB):
                TT = fresh("TT", (128, 1024), dtype=BF16)
                swap_half(GA, TT, 0)
                swap_half(GB, TT, 32)
                return TT

            # ---------- initial state ----------
            Xp = dstate.tile([128, NF], F32, tag="Xp", name="Xp")
            Yp = dstate.tile([128, NF], F32, tag="Yp", name="Yp")
            Zp = dstate.tile([128, NF], F32, tag="Zp", name="Zp")
            nc.vector.tensor_copy(out=Xp[:, :], in_=PTS0[:, 0:NF])
            nc.vector.tensor_copy(out=Yp[:, :], in_=PTS0[:, NF:2 * NF])
            nc.vector.tensor_copy(out=Zp[:, :], in_=PTS0[:, 2 * NF:3 * NF])
            V3 = dstate.tile([128, 3 * NF], F32, tag="V3", name="V3")
            nc.vector.memset(V3[:, :], 0.0)

            BCS0 = state.tile([8, 16], F32)
            nc.vector.memset(BCS0[:, :], 0.0)
            nc.vector.memset(BCS0[:, 9:10], 1.0)
            nc.vector.tensor_copy(out=BCS0[:, 12:14], in_=TV[:, 0:2])
            PS2 = pspool.tile([128, 16], F32, tag="PS2", name="PS2")
            nc.tensor.matmul(PS2[:, :], L2[:, :], BCS0[:, :], start=True,
                             stop=True)

            idx_c, xf_c, yf_c, _ii0 = idx_block(Xp[:, :], Yp[:, :], "0")
            GA_c = gath.tile([128, 4096], BF16, tag="GA", name="GA0")
            GB_c = gath.tile([128, 4096], BF16, tag="GB", name="GB0")
            nc.gpsimd.ap_gather(GA_c[:, :], WIN[:, :], idx_c[:, 0:32],
                                channels=128, num_elems=NE, d=8, num_idxs=512)
            nc.gpsimd.ap_gather(GB_c[:, :], WIN[:, :], idx_c[:, 32:64],
                                channels=128, num_elems=NE, d=8, num_idxs=512)

            cur = dict(Xp=Xp, Yp=Yp, Zp=Zp, V3=V3,
                       xf=xf_c, yf=yf_c, GA=GA_c, GB=GB_c, PS2=PS2, BC=BCS0)

            for t in range(nsteps):
                PS2 = cur["PS2"]
                # SBUF copy of broadcast values (for ACT scalar args)
                BCA = fresh("BCA", (128, 16))
                BCN = fresh("BCN", (128, 16))
                nc.scalar.copy(BCA[:, :], PS2[:, :])
                nc.scalar.activation(BCN[:, :], PS2[:, :], AF.Copy, scale=-1.0)
                sx = [BCA[:, c:c + 1] for c in range(3)]
                sxd = [BCA[:, 3 + c:4 + c] for c in range(3)]
                som = [BCA[:, 6 + c:7 + c] for c in range(3)]
                svc = [[BCA[:, 9 + 3 * tk + c:10 + 3 * tk + c]
                        for c in range(3)] for tk in range(2)]

                # ---- E phase
                RX = fresh("RX"); RY = fresh("RY"); RZ = fresh("RZ")
                nc.scalar.activation(RX[:, :], cur["Xp"][:, :], AF.Identity,
                                     bias=BCN[:, 0:1])
                nc.scalar.activation(RY[:, :], cur["Yp"][:, :], AF.Identity,
                                     bias=BCN[:, 1:2])
                nc.scalar.activation(RZ[:, :], cur["Zp"][:, :], AF.Identity,
                                     bias=BCN[:, 2:3])
                V3n = dstate.tile([128, 3 * NF], F32, tag="V3", name="V3n")
                VXn = V3n[:, 0:NF]
                VYn = V3n[:, NF:2 * NF]
                VZn = V3n[:, 2 * NF:3 * NF]
                e1 = fresh("e1"); e2 = fresh("e2"); e3 = fresh("e3")
                nc.scalar.activation(e1[:, :], RY[:, :], AF.Copy,
                                     scale=som[2])
                h1x = fresh("h1x")
                nc.scalar.activation(h1x[:, :], RZ[:, :], AF.Copy,
                                     scale=som[1])
                nc.vector.tensor_tensor(out=VXn, in0=h1x[:, :],
                                        in1=e1[:, :], op=OP.subtract)
                nc.scalar.activation(e2[:, :], RZ[:, :], AF.Copy,
                                     scale=som[0])
                h1y = fresh("h1y")
                nc.scalar.activation(h1y[:, :], RX[:, :], AF.Copy,
                                     scale=som[2])
                nc.vector.tensor_tensor(out=VYn, in0=h1y[:, :],
                                        in1=e2[:, :], op=OP.subtract)
                nc.scalar.activation(e3[:, :], RX[:, :], AF.Copy,
                                     scale=som[1])
                nc.vector.scalar_tensor_tensor(out=VZn, in0=RY[:, :],
                                               scalar=som[0], in1=e3[:, :],
                                               op0=OP.mult, op1=OP.subtract)
                nc.scalar.activation(VXn, VXn, AF.Identity,
                                     bias=sxd[0])
                nc.scalar.activation(VYn, VYn, AF.Identity,
                                     bias=sxd[1])
                nc.scalar.activation(VZn, VZn, AF.Identity,
                                     bias=sxd[2])
                Xn = dstate.tile([128, NF], F32, tag="Xp", name="Xn")
                Yn = dstate.tile([128, NF], F32, tag="Yp", name="Yn")
                Zn = dstate.tile([128, NF], F32, tag="Zp", name="Zn")
                hxp = fresh("hxp")
                nc.scalar.activation(hxp[:, :], VXn, AF.Copy, scale=C_RK4)
                nc.vector.tensor_tensor(out=Xn[:, :], in0=hxp[:, :],
                                        in1=cur["Xp"][:, :], op=OP.add)
                hyp = fresh("hyp")
                nc.scalar.activation(hyp[:, :], VYn, AF.Copy, scale=C_RK4)
                nc.vector.tensor_tensor(out=Yn[:, :], in0=hyp[:, :],
                                        in1=cur["Yp"][:, :], op=OP.add)
                nc.vector.scalar_tensor_tensor(out=Zn[:, :], in0=VZn,
                                               scalar=C_RK4,
                                               in1=cur["Zp"][:, :],
                                               op0=OP.mult, op1=OP.add)

                if t < nsteps - 1:
                    idx_n, xf_n, yf_n, ii_n = idx_block(Xn[:, :], Yn[:, :], "n")
                    GA_n = gath.tile([128, 4096], BF16, tag="GA", name="GAn")
                    GB_n = gath.tile([128, 4096], BF16, tag="GB", name="GBn")
                    nc.gpsimd.ap_gather(GA_n[:, :], WIN[:, :], idx_n[:, 0:32],
                                        channels=128, num_elems=NE, d=8,
                                        num_idxs=512)
                    nc.gpsimd.ap_gather(GB_n[:, :], WIN[:, :], idx_n[:, 32:64],
                                        channels=128, num_elems=NE, d=8,
                                        num_idxs=512)
                else:
                    xf_n = yf_n = GA_n = GB_n = None
                # (gather stays the only Pool-engine work per step)

                # ---- S phase
                TT = swap_block(cur["GA"], cur["GB"])  # TTF f32
                xf, yf = cur["xf"], cur["yf"]
                xf2 = xf[:, :].rearrange("p (f e) -> p f e", e=1)
                yf2 = yf[:, :].rearrange("p (f e) -> p f e", e=1)

                def quad(g, e):
                    blk = TT[:, 512 * (g // 2):512 * (g // 2) + 512]
                    return blk.rearrange("p (f q) -> p f q", q=8)[
                        :, :, 4 * (g % 2) + e:4 * (g % 2) + e + 1]

                # ---- F phase (grid-batched bilinear + component-packed forces)
                xf, yf = cur["xf"], cur["yf"]
                TTQ = TT[:, :].rearrange("p (k f q) -> p k f q", k=2, q=8)
                vA = TTQ[:, :, :, 0:5:4]
                dxA = TTQ[:, :, :, 1:6:4]
                v1A = TTQ[:, :, :, 2:7:4]
                dx1A = TTQ[:, :, :, 3:8:4]
                xfb = xf[:, :].rearrange("p (a f b) -> p a f b", a=1,
                                         b=1).broadcast_to((128, 2, NF, 2))
                yfb = yf[:, :].rearrange("p (a f b) -> p a f b", a=1,
                                         b=1).broadcast_to((128, 2, NF, 2))
                IA0 = fresh("IA0", (128, 4 * NF))
                IA1 = fresh("IA1", (128, 4 * NF))
                DYA = fresh("DYA", (128, 4 * NF))
                VAL = fresh("VAL", (128, 4 * NF))
                IA0v = IA0[:, :].rearrange("p (k f h) -> p k f h", k=2, h=2)
                IA1v = IA1[:, :].rearrange("p (k f h) -> p k f h", k=2, h=2)
                DYAv = DYA[:, :].rearrange("p (k f h) -> p k f h", k=2, h=2)
                VALv = VAL[:, :].rearrange("p (k f h) -> p k f h", k=2, h=2)
                nc.vector.tensor_tensor(out=IA0v, in0=xfb, in1=dxA, op=OP.mult)
                nc.vector.tensor_tensor(out=IA0v, in0=IA0v, in1=vA, op=OP.add)
                nc.vector.tensor_tensor(out=IA1v, in0=xfb, in1=dx1A,
                                        op=OP.mult)
                nc.vector.tensor_tensor(out=IA1v, in0=IA1v, in1=v1A,
                                        op=OP.add)
                nc.vector.tensor_tensor(out=DYAv, in0=IA1v, in1=IA0v,
                                        op=OP.subtract)
                nc.vector.tensor_tensor(out=VALv, in0=yfb, in1=DYAv,
                                        op=OP.mult)
                nc.vector.tensor_tensor(out=VALv, in0=VALv, in1=IA0v,
                                        op=OP.add)

                def gview(T, g):  # [128, 64] view of grid g in (k,f,h) tile
                    base = 2 * NF * (g // 2) + (g % 2)
                    end = 2 * NF * (g // 2) + 2 * NF
                    return T[:, base:end:2]

                zp = gview(VAL, 0)
                kp = gview(VAL, 1)
                cp = gview(VAL, 2)
                fp = gview(VAL, 3)

                # normals from z-grid diffs
                dx0z = TTQ[:, 0:1, :, 1:2]
                dx1z = TTQ[:, 0:1, :, 3:4]
                dzx = fresh("dzx", (128, NF))
                dzxv = dzx[:, :].rearrange("p (a f b) -> p a f b", a=1, b=1)
                nc.vector.tensor_tensor(out=dzxv, in0=dx1z, in1=dx0z,
                                        op=OP.subtract)
                nc.vector.tensor_tensor(out=dzx[:, :], in0=yf[:, :],
                                        in1=dzx[:, :], op=OP.mult)
                nc.vector.tensor_tensor(out=dzxv, in0=dzxv, in1=dx0z,
                                        op=OP.add)
                q = dzx[:, :]
                p2 = gview(DYA, 0)
                sq1 = fresh("sq1")
                sq2 = fresh("sq2")
                nc.scalar.square(sq1[:, :], q)
                nc.scalar.square(sq2[:, :], p2)
                ss = fresh("ss")
                nc.vector.tensor_tensor(out=ss[:, :], in0=sq1[:, :],
                                        in1=sq2[:, :], op=OP.add)
                nc.vector.tensor_scalar(out=ss[:, :], in0=ss[:, :],
                                        scalar1=100.0, scalar2=1.0,
                                        op0=OP.mult, op1=OP.add)
                rt = fresh("rt")
                nc.scalar.sqrt(rt[:, :], ss[:, :])
                rn = fresh("rn")
                nc.vector.reciprocal(out=rn[:, :], in_=rt[:, :])
                N3 = fresh("N3", (128, 3 * NF))
                nx = N3[:, 0:NF]
                ny = N3[:, NF:2 * NF]
                nz = N3[:, 2 * NF:3 * NF]
                nc.vector.tensor_scalar(out=nx, in0=q, scalar1=-10.0,
                                        scalar2=None, op0=OP.mult)
                nc.vector.tensor_tensor(out=nx, in0=nx, in1=rn[:, :],
                                        op=OP.mult)
                nc.vector.tensor_scalar(out=ny, in0=p2, scalar1=-10.0,
                                        scalar2=None, op0=OP.mult)
                nc.vector.tensor_tensor(out=ny, in0=ny, in1=rn[:, :],
                                        op=OP.mult)
                nc.scalar.copy(nz, rn[:, :])

                def b3(ap2d):  # [128, 64] -> broadcast over 3 comp blocks
                    return ap2d.rearrange("p (a f) -> p a f",
                                          a=1).broadcast_to((128, 3, NF))

                V3c = cur["V3"]
                V3v = V3c[:, :].rearrange("p (c f) -> p c f", c=3)
                N3v = N3[:, :].rearrange("p (c f) -> p c f", c=3)
                dh = fresh("dh")
                nc.vector.tensor_tensor(out=dh[:, :], in0=cur["Zp"][:, :],
                                        in1=zp, op=OP.subtract)
                # xdn = sum_c V3*N3
                M3 = fresh("M3", (128, 3 * NF))
                nc.vector.tensor_tensor(out=M3[:, :], in0=V3c[:, :],
                                        in1=N3[:, :], op=OP.mult)
                xdn = fresh("xdn")
                nc.vector.tensor_reduce(
                    out=xdn[:, :],
                    in_=M3[:, :].rearrange("p (c f) -> p f c", c=3),
                    axis=mybir.AxisListType.X, op=OP.add)

                t1 = fresh("t1")
                t2 = fresh("t2")
                nc.vector.tensor_tensor(out=t1[:, :], in0=kp, in1=dh[:, :],
                                        op=OP.mult)
                nc.vector.tensor_tensor(out=t2[:, :], in0=cp, in1=xdn[:, :],
                                        op=OP.mult)
                nc.vector.tensor_tensor(out=t1[:, :], in0=t1[:, :],
                                        in1=t2[:, :], op=OP.add)
                ctc = fresh("ctc")
                nc.vector.scalar_tensor_tensor(out=ctc[:, :], in0=dh[:, :],
                                               scalar=0.0, in1=NEG[:, :],
                                               op0=OP.is_le, op1=OP.mult)
                Ssc = fresh("Ssc")
                nc.vector.tensor_tensor(out=Ssc[:, :], in0=t1[:, :],
                                        in1=ctc[:, :], op=OP.mult)
                FS3 = fresh("FS3", (128, 3 * NF))
                FS3v = FS3[:, :].rearrange("p (c f) -> p c f", c=3)
                nc.vector.tensor_tensor(out=FS3v, in0=b3(Ssc[:, :]),
                                        in1=N3v, op=OP.mult)
                Nm = fresh("Nm")
                nc.scalar.activation(Nm[:, :], Ssc[:, :], AF.Abs)
                fN = fresh("fN")
                nc.vector.tensor_tensor(out=fN[:, :], in0=fp, in1=Nm[:, :],
                                        op=OP.mult)

                # thrust projection shared across tracks:
                # dv_tau^tk = s_tk * P + Q,  P = td - (td.n) n, Q = xdn*n - V
                tn = fresh("tn")
                nc.vector.tensor_scalar(out=tn[:, :], in0=nx,
                                        scalar1=svc[0][0], scalar2=None,
                                        op0=OP.mult)
                nc.vector.scalar_tensor_tensor(out=tn[:, :], in0=ny,
                                               scalar=svc[0][1],
                                               in1=tn[:, :],
                                               op0=OP.mult, op1=OP.add)
                nc.vector.scalar_tensor_tensor(out=tn[:, :], in0=nz,
                                               scalar=svc[0][2],
                                               in1=tn[:, :],
                                               op0=OP.mult, op1=OP.add)
                TD3 = fresh("TD3", (128, 3 * NF))
                for c in range(3):
                    nc.vector.tensor_scalar(out=TD3[:, c * NF:(c + 1) * NF],
                                            in0=ONES[:, :],
                                            scalar1=svc[0][c], scalar2=None,
                                            op0=OP.mult)
                P3 = fresh("P3", (128, 3 * NF))
                P3v = P3[:, :].rearrange("p (c f) -> p c f", c=3)
                nc.vector.tensor_tensor(out=P3v, in0=b3(tn[:, :]), in1=N3v,
                                        op=OP.mult)
                nc.vector.tensor_tensor(out=P3[:, :], in0=TD3[:, :],
                                        in1=P3[:, :], op=OP.subtract)
                Q3 = fresh("Q3", (128, 3 * NF))
                Q3v = Q3[:, :].rearrange("p (c f) -> p c f", c=3)
                nc.vector.tensor_tensor(out=Q3v, in0=b3(xdn[:, :]), in1=N3v,
                                        op=OP.mult)
                nc.vector.tensor_tensor(out=Q3[:, :], in0=Q3[:, :],
                                        in1=V3c[:, :], op=OP.subtract)

                FF3 = fresh("FF3", (128, 3 * NF))
                for tk in range(2):
                    dvt = fresh(f"dvt{tk}", (128, 3 * NF))
                    nc.vector.scalar_tensor_tensor(out=dvt[:, :],
                                                   in0=P3[:, :],
                                                   scalar=svc[1][tk],
                                                   in1=Q3[:, :],
                                                   op0=OP.mult, op1=OP.add)
                    th3 = fresh(f"th3{tk}", (128, 3 * NF))
                    nc.scalar.activation(th3[:, :], dvt[:, :], AF.Tanh)
                    fNt = fresh(f"fNt{tk}")
                    nc.vector.tensor_tensor(out=fNt[:, :], in0=fN[:, :],
                                            in1=MK[:, tk * NF:(tk + 1) * NF],
                                            op=OP.mult)
                    th3v = th3[:, :].rearrange("p (c f) -> p c f", c=3)
                    if tk == 0:
                        FF3v = FF3[:, :].rearrange("p (c f) -> p c f", c=3)
                        nc.vector.tensor_tensor(out=FF3v, in0=th3v,
                                                in1=b3(fNt[:, :]),
                                                op=OP.mult)
                    else:
                        g3 = fresh("g3", (128, 3 * NF))
                        g3v = g3[:, :].rearrange("p (c f) -> p c f", c=3)
                        nc.vector.tensor_tensor(out=g3v, in0=th3v,
                                                in1=b3(fNt[:, :]),
                                                op=OP.mult)
                        nc.vector.tensor_tensor(out=FF3[:, :],
                                                in0=FF3[:, :],
                                                in1=g3[:, :], op=OP.add)

                P6 = fresh("P6", (128, 8))
                F3 = fresh("F3", (128, 3 * NF))
                for c in range(3):
                    nc.vector.scalar_tensor_tensor(
                        out=F3[:, c * NF:(c + 1) * NF],
                        in0=FS3[:, c * NF:(c + 1) * NF], scalar=0.0,
                        in1=FF3[:, c * NF:(c + 1) * NF],
                        op0=OP.bypass, op1=OP.add,
                        accum_out=P6[:, c:c + 1])
                Fx = F3[:, 0:NF]
                Fy = F3[:, NF:2 * NF]
                Fz = F3[:, 2 * NF:3 * NF]
                tta = fresh("tta"); ttb = fresh("ttb")
                ttc = fresh("ttc"); ttd = fresh("ttd")
                tte = fresh("tte"); ttf = fresh("ttf")
                nc.vector.tensor_tensor(out=tta[:, :], in0=RY[:, :],
                                        in1=Fz, op=OP.mult)
                nc.vector.tensor_tensor(out=ttb[:, :], in0=RZ[:, :],
                                        in1=Fy, op=OP.mult)
                tx = fresh("tx")
                nc.vector.scalar_tensor_tensor(out=tx[:, :], in0=tta[:, :],
                                               scalar=0.0, in1=ttb[:, :],
                                               op0=OP.bypass, op1=OP.subtract,
                                               accum_out=P6[:, 3:4])
                nc.vector.tensor_tensor(out=ttc[:, :], in0=RZ[:, :],
                                        in1=Fx, op=OP.mult)
                nc.vector.tensor_tensor(out=ttd[:, :], in0=RX[:, :],
                                        in1=Fz, op=OP.mult)
                ty = fresh("ty")
                nc.vector.scalar_tensor_tensor(out=ty[:, :], in0=ttc[:, :],
                                               scalar=0.0, in1=ttd[:, :],
                                               op0=OP.bypass, op1=OP.subtract,
                                               accum_out=P6[:, 4:5])
                nc.vector.tensor_tensor(out=tte[:, :], in0=RX[:, :],
                                        in1=Fy, op=OP.mult)
                nc.vector.tensor_tensor(out=ttf[:, :], in0=RY[:, :],
                                        in1=Fx, op=OP.mult)
                tz = fresh("tz")
                nc.vector.scalar_tensor_tensor(out=tz[:, :], in0=tte[:, :],
                                               scalar=0.0, in1=ttf[:, :],
                                               op0=OP.bypass, op1=OP.subtract,
                                               accum_out=P6[:, 5:6])

                # ---- R phase
                PS1 = pspool.tile([8, 8], F32, tag="PS1", name="PS1")
                nc.tensor.matmul(PS1[:, 0:6], L1[:, :], P6[:, 0:6],
                                 start=True, stop=True)
                BCn = fresh("BCn", (8, 16))
                omd = fresh("omd", (8, 3))
                nc.vector.tensor_tensor(out=omd[:, :], in0=PS1[:, 3:6],
                                        in1=SCT[:, 0:3], op=OP.mult)
                nc.vector.tensor_scalar(out=omd[:, :], in0=omd[:, :],
                                        scalar1=OMEGA_MAX, scalar2=-OMEGA_MAX,
                                        op0=OP.min, op1=OP.max)
                xdd = fresh("xdd", (8, 3))
                nc.vector.scalar_tensor_tensor(out=xdd[:, :], in0=PS1[:, 0:3],
                                               scalar=1.0 / MASS,
                                               in1=SCT[:, 3:6],
                                               op0=OP.mult, op1=OP.add)
                BCp = cur["BC"]
                nc.vector.scalar_tensor_tensor(out=BCn[:, 3:6],
                                               in0=xdd[:, :], scalar=C_RK4,
                                               in1=BCp[:, 3:6],
                                               op0=OP.mult, op1=OP.add)
                nc.vector.scalar_tensor_tensor(out=BCn[:, 0:3],
                                               in0=BCn[:, 3:6], scalar=C_RK4,
                                               in1=BCp[:, 0:3],
                                               op0=OP.mult, op1=OP.add)
                nc.vector.scalar_tensor_tensor(out=BCn[:, 6:9],
                                               in0=omd[:, :], scalar=C_RK4,
                                               in1=BCp[:, 6:9],
                                               op0=OP.mult, op1=OP.add)
                nc.scalar.copy(XH[:, 3 * t:3 * t + 3], BCn[:, 0:3])

                # small-angle exact-in-f32 Taylor: s2 = (th*DT)^2 <= 0.015
                # a = DT*(1 + s2*(s2/120 - 1/6)); b = DT^2*(0.5 + s2*(s2/720 - 1/24))
                th2 = fresh("th2", (8, 1))
                sqw = fresh("sqw", (8, 3))
                nc.scalar.activation(sqw[:, :], BCn[:, 6:9], AF.Square,
                                     accum_out=th2[:, :])
                s2 = fresh("s2", (8, 1))
                nc.vector.tensor_scalar(out=s2[:, :], in0=th2[:, :],
                                        scalar1=DT * DT, scalar2=None,
                                        op0=OP.mult)
                ua = fresh("ua", (8, 1))
                nc.vector.tensor_scalar(out=ua[:, :], in0=s2[:, :],
                                        scalar1=1.0 / 120.0,
                                        scalar2=-1.0 / 6.0,
                                        op0=OP.mult, op1=OP.add)
                av = fresh("av", (8, 1))
                nc.vector.scalar_tensor_tensor(out=av[:, :], in0=ua[:, :],
                                               scalar=s2[:, :],
                                               in1=SCT[:, 6:7],
                                               op0=OP.mult, op1=OP.add)
                nc.vector.tensor_scalar(out=av[:, :], in0=av[:, :],
                                        scalar1=DT, scalar2=None, op0=OP.mult)
                ub = fresh("ub", (8, 1))
                nc.vector.tensor_scalar(out=ub[:, :], in0=s2[:, :],
                                        scalar1=1.0 / 720.0,
                                        scalar2=-1.0 / 24.0,
                                        op0=OP.mult, op1=OP.add)
                bv = fresh("bv", (8, 1))
                nc.vector.scalar_tensor_tensor(out=bv[:, :], in0=ub[:, :],
                                               scalar=s2[:, :],
                                               in1=SCT[:, 7:8],
                                               op0=OP.mult, op1=OP.add)
                nc.vector.tensor_scalar(out=bv[:, :], in0=bv[:, :],
                                        scalar1=DT * DT, scalar2=None,
                                        op0=OP.mult)
                MM = fresh("MM", (8, 9))
                u = [BCn[:, 6 + c:7 + c] for c in range(3)]
                dd = fresh("dd", (8, 3))
                nc.vector.tensor_tensor(out=dd[:, :], in0=BCn[:, 6:9],
                                        in1=BCn[:, 6:9], op=OP.mult)
                nc.vector.tensor_scalar(out=dd[:, :], in0=dd[:, :],
                                        scalar1=th2[:, :], scalar2=None,
                                        op0=OP.subtract)
                nc.vector.tensor_scalar(out=dd[:, :], in0=dd[:, :],
                                        scalar1=bv[:, :], scalar2=None,
                                        op0=OP.mult)
                nc.vector.tensor_scalar(out=MM[:, 0:9:4], in0=dd[:, :],
                                        scalar1=1.0, scalar2=None, op0=OP.add)
                for (i, j, kk, sgn) in ((0, 1, 2, +1), (0, 2, 1, -1),
                                        (1, 2, 0, +1)):
                    hp = fresh(f"hp{i}{j}", (8, 1))
                    hq = fresh(f"hq{i}{j}", (8, 1))
                    nc.vector.tensor_tensor(out=hp[:, :], in0=u[i], in1=u[j],
                                            op=OP.mult)
                    nc.vector.tensor_scalar(out=hp[:, :], in0=hp[:, :],
                                            scalar1=bv[:, :], scalar2=None,
                                            op0=OP.mult)
                    nc.vector.tensor_tensor(out=hq[:, :], in0=u[kk],
                                            in1=av[:, :], op=OP.mult)
                    a_ij = MM[:, 3 * i + j:3 * i + j + 1]
                    a_ji = MM[:, 3 * j + i:3 * j + i + 1]
                    if sgn > 0:
                        nc.vector.tensor_tensor(out=a_ij, in0=hp[:, :],
                                                in1=hq[:, :], op=OP.subtract)
                        nc.vector.tensor_tensor(out=a_ji, in0=hp[:, :],
                                                in1=hq[:, :], op=OP.add)
                    else:
                        nc.vector.tensor_tensor(out=a_ij, in0=hp[:, :],
                                                in1=hq[:, :], op=OP.add)
                        nc.vector.tensor_tensor(out=a_ji, in0=hp[:, :],
                                                in1=hq[:, :], op=OP.subtract)
                Rn = fresh("Rn", (8, 9))
                for b in range(3):
                    nc.vector.tensor_scalar(out=Rn[:, b:9:3],
                                            in0=RB[:, 0:9:3],
                                            scalar1=MM[:, b:b + 1],
                                            scalar2=None, op0=OP.mult)
                    nc.vector.scalar_tensor_tensor(out=Rn[:, b:9:3],
                                                   in0=RB[:, 1:9:3],
                                                   scalar=MM[:, 3 + b:4 + b],
                                                   in1=Rn[:, b:9:3],
                                                   op0=OP.mult, op1=OP.add)
                    nc.vector.scalar_tensor_tensor(out=Rn[:, b:9:3],
                                                   in0=RB[:, 2:9:3],
                                                   scalar=MM[:, 6 + b:7 + b],
                                                   in1=Rn[:, b:9:3],
                                                   op0=OP.mult, op1=OP.add)
                nc.vector.tensor_copy(out=RB[:, 0:9], in_=Rn[:, 0:9])
                if t < nsteps - 1:
                    ssr = fresh("ssr", (8, 1))
                    sqr = fresh("sqr", (8, 3))
                    nc.scalar.activation(sqr[:, :], Rn[:, 0:9:3], AF.Square,
                                         accum_out=ssr[:, :])
                    rsr = fresh("rsr", (8, 1))
                    nc.vector.tensor_scalar(out=rsr[:, :], in0=ssr[:, :],
                                            scalar1=-0.5, scalar2=1.5,
                                            op0=OP.mult, op1=OP.add)
                    nc.vector.tensor_scalar(out=BCn[:, 9:12],
                                            in0=Rn[:, 0:9:3],
                                            scalar1=rsr[:, :], scalar2=None,
                                            op0=OP.mult)
                    nc.vector.tensor_copy(out=BCn[:, 12:14],
                                          in_=TV[:, 2 * t + 2:2 * t + 4])
                    nc.vector.memset(BCn[:, 14:16], 0.0)
                    PS2n = pspool.tile([128, 16], F32, tag="PS2", name="PS2n")
                    nc.tensor.matmul(PS2n[:, :], L2[:, :], BCn[:, :],
                                     start=True, stop=True)
                else:
                    PS2n = None

                cur = dict(Xp=Xn, Yp=Yn, Zp=Zn, V3=V3n,
                           xf=xf_n, yf=yf_n, GA=GA_n, GB=GB_n, PS2=PS2n,
                           BC=BCn)

            nc.sync.dma_start(out=out_d[:, :], in_=XH[:, :])

    nc.compile()
    return nc


def prep_core_inputs(z_grid, stiffness, damping, friction, controls,
                     x_points0, track_ids, core):
    r0 = core * 8
    import ml_dtypes
    grids = [z_grid, stiffness, damping, friction]
    wins = np.zeros((128, 8 * NE), ml_dtypes.bfloat16)
    for r in range(8):
        for lane in range(2):
            octw = np.empty((WN, WN, 8), np.float32)
            for h in range(2):
                G = np.asarray(grids[2 * lane + h][r0 + r], np.float32)
                sub = G[WX0:WX0 + WN + 1, WY0:WY0 + WN + 1].astype(
                    ml_dtypes.bfloat16)
                subf = sub.astype(np.float32)
                octw[:, :, 4 * h + 0] = subf[0:WN, 0:WN]
                octw[:, :, 4 * h + 1] = subf[1:WN + 1, 0:WN] - subf[0:WN, 0:WN]
                octw[:, :, 4 * h + 2] = subf[0:WN, 1:WN + 1]
                octw[:, :, 4 * h + 3] = subf[1:WN + 1, 1:WN + 1] - \
                    subf[0:WN, 1:WN + 1]
            wins[16 * r + lane, :] = octw.reshape(-1).astype(ml_dtypes.bfloat16)
    pts = np.zeros((128, 3 * NF), np.float32)
    for r in range(8):
        P = np.asarray(x_points0[r0 + r], np.float32)
        for c in range(3):
            pts[16 * r:16 * r + 16, c * NF:(c + 1) * NF] = \
                P[:, c].reshape(NF, 16).T
    msk = np.zeros((128, 2 * NF), np.float32)
    tid = np.asarray(track_ids)
    for tk in range(2):
        blk = (tid == tk).astype(np.float32).reshape(NF, 16).T
        for r in range(8):
            msk[16 * r:16 * r + 16, tk * NF:(tk + 1) * NF] = blk
    tv = np.zeros((8, 2 * T_STEPS), np.float32)
    ctl = np.asarray(controls, np.float32)
    v = ctl[:, r0:r0 + 8, 0]
    w = ctl[:, r0:r0 + 8, 1]
    tv[:, 0::2] = (v - w * ROBOT_LY / 2.0).T
    tv[:, 1::2] = (v + w * ROBOT_LY / 2.0).T
    l1 = np.zeros((128, 8), np.float32)
    for p in range(128):
        l1[p, p // 16] = 1.0
    l2 = np.ascontiguousarray(l1.T)
    sc = np.zeros((8, 8), np.float32)
    sc[:, 0:3] = [1.0, 1.0 / 3.5, 1.0 / 4.0]
    sc[:, 5] = -GRAV
    sc[:, 6] = 1.0
    sc[:, 7] = 0.5
    return dict(wins=wins, pts=pts, msk=msk, tv=tv, l1=l1, l2=l2, sc=sc)


def postprocess(results):
    out = np.zeros((T_STEPS, 64, 3), np.float32)
    for core in range(8):
        o = np.asarray(results[core]["out"])
        for r in range(8):
            out[:, core * 8 + r, :] = o[r].reshape(T_STEPS, 3)
    return out


# ----------------------------------------------------------------------------
# Harness entry point: full inputs in, full output out.
# ----------------------------------------------------------------------------
_NC_CACHE = {}


def kernel(z_grid, stiffness, damping, friction, controls, x_points0,
           track_ids):
    import numpy as np
    from concourse.bass_utils import run_bass_kernel_spmd

    z_grid = np.asarray(z_grid, np.float32)
    stiffness = np.asarray(stiffness, np.float32)
    damping = np.asarray(damping, np.float32)
    friction = np.asarray(friction, np.float32)
    controls = np.asarray(controls, np.float32)
    x_points0 = np.asarray(x_points0, np.float32)
    track_ids = np.asarray(track_ids, np.int32)

    if "nc" not in _NC_CACHE:
        _NC_CACHE["nc"] = build(nsteps=T_STEPS)
    nc = _NC_CACHE["nc"]

    in_maps = [prep_core_inputs(z_grid, stiffness, damping, friction,
                                controls, x_points0, track_ids, core)
               for core in range(8)]
    res = run_bass_kernel_spmd(nc, in_maps, core_ids=list(range(8)))
    return postprocess(res.results)



# revision 9
# speedup vs baseline: 2.4022x; 2.4022x over previous
"""Distributed Bass kernel for the DPhysics problem (8 TRN2 NeuronCores).

Layout: 8 robots/core; compute tiles [128, 64]: partition 16r+j, free f;
point i of robot r at (16r + i%16, i//16).
Terrain: GPSIMD ap_gather (d=2 pair windows, 128x128 cells, +-6.4 m).
Window partition 16r + 2g+s holds pairs (value, x-diff) of grid g, y-shift s.
Swap to compute layout: 128 masked stream_shuffles (DVE, mask=255 suppress).
Reductions/broadcasts: PE matmuls with one-hot block matrices.
"""

import numpy as np
import concourse.bass as bass
import concourse.bacc as bacc
import concourse.mybir as mybir
from concourse.tile import TileContext
from bass_rust import add_dep_helper

F32 = mybir.dt.float32
I32 = mybir.dt.int32
I16 = mybir.dt.int16
AF = mybir.ActivationFunctionType
OP = mybir.AluOpType

D_MAX = 12.8
RES = 0.1
DT = 0.01
MASS = 40.0
GRAV = 9.8
ROBOT_LY = 0.5
OMEGA_MAX = 7.0
T_STEPS = 64
NPTS = 1024
NF = 64
C_RK4 = DT * (1.0 + DT / 2.0 + DT * DT / 6.0 + DT ** 3 / 24.0)
WX0 = 112
WY0 = 112
WN = 32
NE = WN * WN
CLIP_LO = float(WX0)
CLIP_HI = WX0 + WN - 1.0 / 32.0
BF16 = mybir.dt.bfloat16
W_SUB = WN * WX0 + WY0  # offset subtracted from WN*xi+yi


def build(nsteps=T_STEPS, sim_safe_masks=False, debug=False,
          gps_eff=None):
    if gps_eff is not None:
        from concourse import hw_specs
        hw_specs.TRN2Spec.GPSIMD_IMPL_EFFICIENCY_DEFAULT = gps_eff
    nc = bacc.Bacc(target_bir_lowering=False)

    wins_d = nc.declare_dram_parameter("wins", [128, 8 * NE], BF16,
                                       isOutput=False)
    pts_d = nc.declare_dram_parameter("pts", [128, 3 * NF], F32, isOutput=False)
    msk_d = nc.declare_dram_parameter("msk", [128, 2 * NF], F32, isOutput=False)
    tv_d = nc.declare_dram_parameter("tv", [8, 2 * T_STEPS], F32, isOutput=False)
    l1_d = nc.declare_dram_parameter("l1", [128, 8], F32, isOutput=False)
    l2_d = nc.declare_dram_parameter("l2", [8, 128], F32, isOutput=False)
    sc_d = nc.declare_dram_parameter("sc", [8, 8], F32, isOutput=False)
    out_d = nc.declare_dram_parameter("out", [8, 3 * T_STEPS], F32, isOutput=True)
    if debug:
        dbg_d = nc.declare_dram_parameter("dbg", [128, 16 * NF], F32,
                                          isOutput=True)

    with TileContext(nc) as tc:
        with (
            tc.tile_pool(name="big", bufs=1) as big,
            tc.tile_pool(name="state", bufs=1) as state,
            tc.tile_pool(name="dstate", bufs=2) as dstate,
            tc.tile_pool(name="gath", bufs=2) as gath,
            tc.tile_pool(name="tmp", bufs=2) as tmp,
            tc.tile_pool(name="ps", bufs=2, space="PSUM") as pspool,
        ):
            WIN = big.tile([128, 8 * NE], BF16)
            MK = state.tile([128, 2 * NF], F32)
            TV = state.tile([8, 2 * T_STEPS], F32)
            L1 = state.tile([128, 8], F32)
            L2 = state.tile([8, 128], F32)
            SCT = state.tile([8, 8], F32)
            NEG = state.tile([128, NF], F32)
            RB = state.tile([8, 12], F32)
            XH = state.tile([8, 3 * T_STEPS], F32)
            PTS0 = state.tile([128, 3 * NF], F32)

            nc.sync.dma_start(out=WIN[:, :], in_=wins_d[:, :])
            nc.sync.dma_start(out=PTS0[:, :], in_=pts_d[:, :])
            nc.sync.dma_start(out=MK[:, :], in_=msk_d[:, :])
            nc.sync.dma_start(out=TV[:, :], in_=tv_d[:, :])
            nc.sync.dma_start(out=L1[:, :], in_=l1_d[:, :])
            nc.sync.dma_start(out=L2[:, :], in_=l2_d[:, :])
            nc.sync.dma_start(out=SCT[:, :], in_=sc_d[:, :])

            CB128 = state.tile([128, 1], F32)
            CPI2 = state.tile([128, 1], F32)
            Cm96 = state.tile([128, 1], F32)
            C6397 = state.tile([128, 1], F32)
            C15997 = state.tile([128, 1], F32)
            Cm6240 = state.tile([128, 1], F32)
            Cm05 = state.tile([128, 1], F32)
            nc.vector.memset(CB128[:, :], 128.0)
            nc.vector.memset(CPI2[:, :], float(np.pi / 2))
            nc.vector.memset(Cm96[:, :], -CLIP_LO)
            nc.vector.memset(C6397[:, :], CLIP_HI - CLIP_LO)
            nc.vector.memset(C15997[:, :], CLIP_HI)
            nc.vector.memset(Cm6240[:, :], -float(W_SUB))
            nc.vector.memset(Cm05[:, :], -0.5)
            ONES = state.tile([128, NF], F32)
            nc.vector.memset(ONES[:, :], 1.0)
            nc.vector.memset(NEG[:, :], -1.0 / NPTS)
            nc.vector.memset(RB[:, 0:12], 0.0)
            nc.vector.memset(RB[:, 0:1], 1.0)
            nc.vector.memset(RB[:, 4:5], 1.0)
            nc.vector.memset(RB[:, 8:9], 1.0)

            def fresh(name, shape=(128, NF), dtype=F32):
                return tmp.tile(list(shape), dtype, name=name)

            def idx_block(Xc, Yc, tag):
                # xs = clip(10*X + 128, 96, 159.96875) via Relu chain (ACT)
                xs = fresh("xs" + tag)
                ys = fresh("ys" + tag)
                for (o, i_) in ((xs, Xc), (ys, Yc)):
                    nc.scalar.activation(o[:, :], i_, AF.Identity,
                                         bias=CB128[:, 0:1], scale=10.0)
                    nc.scalar.activation(o[:, :], o[:, :], AF.Relu,
                                         bias=Cm96[:, 0:1])
                    nc.scalar.activation(o[:, :], o[:, :], AF.Relu,
                                         bias=C6397[:, 0:1], scale=-1.0)
                    nc.scalar.activation(o[:, :], o[:, :], AF.Identity,
                                         bias=C15997[:, 0:1], scale=-1.0)
                # floor via round(x - 0.5) (exact-int x gives same interp)
                xsh = fresh("xsh" + tag)
                ysh = fresh("ysh" + tag)
                nc.scalar.activation(xsh[:, :], xs[:, :], AF.Identity,
                                     bias=Cm05[:, 0:1])
                nc.scalar.activation(ysh[:, :], ys[:, :], AF.Identity,
                                     bias=Cm05[:, 0:1])
                xi = fresh("xi" + tag, dtype=I32)
                yi = fresh("yi" + tag, dtype=I32)
                xif = fresh("xif" + tag)
                yif = fresh("yif" + tag)
                nc.scalar.copy(xi[:, :], xsh[:, :])
                nc.scalar.copy(yi[:, :], ysh[:, :])
                nc.scalar.copy(xif[:, :], xi[:, :])
                nc.scalar.copy(yif[:, :], yi[:, :])
                xf = fresh("xf" + tag)
                yf = fresh("yf" + tag)
                nc.vector.tensor_tensor(out=xf[:, :], in0=xs[:, :],
                                        in1=xif[:, :], op=OP.subtract)
                nc.vector.tensor_tensor(out=yf[:, :], in0=ys[:, :],
                                        in1=yif[:, :], op=OP.subtract)
                # w = WN*xif + yif - W_SUB
                wh = fresh("wh" + tag)
                nc.scalar.activation(wh[:, :], xif[:, :], AF.Identity,
                                     bias=Cm6240[:, 0:1], scale=float(WN))
                wf = fresh("wf" + tag)
                nc.vector.tensor_tensor(out=wf[:, :], in0=wh[:, :],
                                        in1=yif[:, :], op=OP.add)
                idx = fresh("idx" + tag, dtype=I16)
                ii = nc.scalar.copy(idx[:, :], wf[:, :])
                return idx, xf, yf, ii

            def swap_block(G):
                # G: [128, 8192] bf16 = 1024 idx x oct; slot m = 16f + v
                # holds (robot r, point row v, col f) data in partition
                # 16r + k (k = lane).  Writes TT all 64 f-cols of both
                # variant blocks.
                TT = fresh("TT", (128, 1024), dtype=BF16)
                Gf = G[:, :].bitcast(F32)      # [128, 4096] f32 words
                Gv = Gf.rearrange("p (m e) -> p m e", e=4)
                TTf = TT[:, :].bitcast(F32)
                for k in range(2):
                    dst = TTf[:, 256 * k:256 * (k + 1)].rearrange(
                        "p (f e) -> p f e", e=4)
                    for v in range(16):
                        if sim_safe_masks:
                            mask = [16 * (u // 16) + k for u in range(32)]
                        else:
                            mask = [255] * 32
                            mask[v] = k
                            mask[16 + v] = 16 + k
                        nc.vector.stream_shuffle(
                            out=dst, in_=Gv[:, v:1024:16, :], mask=mask)
                return TT

            # ---------- initial state ----------
            Xp = dstate.tile([128, NF], F32, tag="Xp", name="Xp")
            Yp = dstate.tile([128, NF], F32, tag="Yp", name="Yp")
            Zp = dstate.tile([128, NF], F32, tag="Zp", name="Zp")
            nc.vector.tensor_copy(out=Xp[:, :], in_=PTS0[:, 0:NF])
            nc.vector.tensor_copy(out=Yp[:, :], in_=PTS0[:, NF:2 * NF])
            nc.vector.tensor_copy(out=Zp[:, :], in_=PTS0[:, 2 * NF:3 * NF])
            V3 = dstate.tile([128, 3 * NF], F32, tag="V3", name="V3")
            nc.vector.memset(V3[:, :], 0.0)

            BCS0 = state.tile([8, 16], F32)
            nc.vector.memset(BCS0[:, :], 0.0)
            nc.vector.memset(BCS0[:, 9:10], 1.0)
            nc.vector.tensor_copy(out=BCS0[:, 12:14], in_=TV[:, 0:2])
            PS2 = pspool.tile([128, 16], F32, tag="PS2", name="PS2")
            nc.tensor.matmul(PS2[:, :], L2[:, :], BCS0[:, :], start=True,
                             stop=True)

            idx_c, xf_c, yf_c, _ii0 = idx_block(Xp[:, :], Yp[:, :], "0")
            GA_c = gath.tile([128, 8192], BF16, tag="GA", name="GA0")
            nc.gpsimd.ap_gather(GA_c[:, :], WIN[:, :], idx_c[:, 0:64],
                                channels=128, num_elems=NE, d=8, num_idxs=1024)

            cur = dict(Xp=Xp, Yp=Yp, Zp=Zp, V3=V3,
                       xf=xf_c, yf=yf_c, GA=GA_c, PS2=PS2, BC=BCS0)

            for t in range(nsteps):
                PS2 = cur["PS2"]
                # SBUF copy of broadcast values (for ACT scalar args)
                BCA = fresh("BCA", (128, 16))
                BCN = fresh("BCN", (128, 16))
                nc.scalar.copy(BCA[:, :], PS2[:, :])
                nc.scalar.activation(BCN[:, :], PS2[:, :], AF.Copy, scale=-1.0)
                sx = [BCA[:, c:c + 1] for c in range(3)]
                sxd = [BCA[:, 3 + c:4 + c] for c in range(3)]
                som = [BCA[:, 6 + c:7 + c] for c in range(3)]
                svc = [[BCA[:, 9 + 3 * tk + c:10 + 3 * tk + c]
                        for c in range(3)] for tk in range(2)]

                # ---- E phase
                RX = fresh("RX"); RY = fresh("RY"); RZ = fresh("RZ")
                nc.scalar.activation(RX[:, :], cur["Xp"][:, :], AF.Identity,
                                     bias=BCN[:, 0:1])
                nc.scalar.activation(RY[:, :], cur["Yp"][:, :], AF.Identity,
                                     bias=BCN[:, 1:2])
                nc.scalar.activation(RZ[:, :], cur["Zp"][:, :], AF.Identity,
                                     bias=BCN[:, 2:3])
                V3n = dstate.tile([128, 3 * NF], F32, tag="V3", name="V3n")
                VXn = V3n[:, 0:NF]
                VYn = V3n[:, NF:2 * NF]
                VZn = V3n[:, 2 * NF:3 * NF]
                e1 = fresh("e1"); e2 = fresh("e2"); e3 = fresh("e3")
                nc.scalar.activation(e1[:, :], RY[:, :], AF.Copy,
                                     scale=som[2])
                h1x = fresh("h1x")
                nc.scalar.activation(h1x[:, :], RZ[:, :], AF.Copy,
                                     scale=som[1])
                nc.vector.tensor_tensor(out=VXn, in0=h1x[:, :],
                                        in1=e1[:, :], op=OP.subtract)
                nc.scalar.activation(e2[:, :], RZ[:, :], AF.Copy,
                                     scale=som[0])
                h1y = fresh("h1y")
                nc.scalar.activation(h1y[:, :], RX[:, :], AF.Copy,
                                     scale=som[2])
                nc.vector.tensor_tensor(out=VYn, in0=h1y[:, :],
                                        in1=e2[:, :], op=OP.subtract)
                nc.scalar.activation(e3[:, :], RX[:, :], AF.Copy,
                                     scale=som[1])
                nc.vector.scalar_tensor_tensor(out=VZn, in0=RY[:, :],
                                               scalar=som[0], in1=e3[:, :],
                                               op0=OP.mult, op1=OP.subtract)
                nc.scalar.activation(VXn, VXn, AF.Identity,
                                     bias=sxd[0])
                nc.scalar.activation(VYn, VYn, AF.Identity,
                                     bias=sxd[1])
                nc.scalar.activation(VZn, VZn, AF.Identity,
                                     bias=sxd[2])
                Xn = dstate.tile([128, NF], F32, tag="Xp", name="Xn")
                Yn = dstate.tile([128, NF], F32, tag="Yp", name="Yn")
                Zn = dstate.tile([128, NF], F32, tag="Zp", name="Zn")
                hxp = fresh("hxp")
                nc.scalar.activation(hxp[:, :], VXn, AF.Copy, scale=C_RK4)
                nc.vector.tensor_tensor(out=Xn[:, :], in0=hxp[:, :],
                                        in1=cur["Xp"][:, :], op=OP.add)
                hyp = fresh("hyp")
                nc.scalar.activation(hyp[:, :], VYn, AF.Copy, scale=C_RK4)
                nc.vector.tensor_tensor(out=Yn[:, :], in0=hyp[:, :],
                                        in1=cur["Yp"][:, :], op=OP.add)
                nc.vector.scalar_tensor_tensor(out=Zn[:, :], in0=VZn,
                                               scalar=C_RK4,
                                               in1=cur["Zp"][:, :],
                                               op0=OP.mult, op1=OP.add)

                if t < nsteps - 1:
                    idx_n, xf_n, yf_n, ii_n = idx_block(Xn[:, :], Yn[:, :], "n")
                    GA_n = gath.tile([128, 8192], BF16, tag="GA", name="GAn")
                    nc.gpsimd.ap_gather(GA_n[:, :], WIN[:, :], idx_n[:, 0:64],
                                        channels=128, num_elems=NE, d=8,
                                        num_idxs=1024)
                else:
                    xf_n = yf_n = GA_n = None
                # (gather stays the only Pool-engine work per step)

                # ---- S phase
                TT = swap_block(cur["GA"])  # TTF f32
                xf, yf = cur["xf"], cur["yf"]
                xf2 = xf[:, :].rearrange("p (f e) -> p f e", e=1)
                yf2 = yf[:, :].rearrange("p (f e) -> p f e", e=1)

                def quad(g, e):
                    blk = TT[:, 512 * (g // 2):512 * (g // 2) + 512]
                    return blk.rearrange("p (f q) -> p f q", q=8)[
                        :, :, 4 * (g % 2) + e:4 * (g % 2) + e + 1]

                # ---- F phase (grid-batched bilinear + component-packed forces)
                xf, yf = cur["xf"], cur["yf"]
                TTQ = TT[:, :].rearrange("p (k f q) -> p k f q", k=2, q=8)
                vA = TTQ[:, :, :, 0:5:4]
                dxA = TTQ[:, :, :, 1:6:4]
                v1A = TTQ[:, :, :, 2:7:4]
                dx1A = TTQ[:, :, :, 3:8:4]
                xfb = xf[:, :].rearrange("p (a f b) -> p a f b", a=1,
                                         b=1).broadcast_to((128, 2, NF, 2))
                yfb = yf[:, :].rearrange("p (a f b) -> p a f b", a=1,
                                         b=1).broadcast_to((128, 2, NF, 2))
                IA0 = fresh("IA0", (128, 4 * NF))
                IA1 = fresh("IA1", (128, 4 * NF))
                DYA = fresh("DYA", (128, 4 * NF))
                VAL = fresh("VAL", (128, 4 * NF))
                IA0v = IA0[:, :].rearrange("p (k f h) -> p k f h", k=2, h=2)
                IA1v = IA1[:, :].rearrange("p (k f h) -> p k f h", k=2, h=2)
                DYAv = DYA[:, :].rearrange("p (k f h) -> p k f h", k=2, h=2)
                VALv = VAL[:, :].rearrange("p (k f h) -> p k f h", k=2, h=2)
                nc.vector.tensor_tensor(out=IA0v, in0=xfb, in1=dxA, op=OP.mult)
                nc.vector.tensor_tensor(out=IA0v, in0=IA0v, in1=vA, op=OP.add)
                nc.vector.tensor_tensor(out=IA1v, in0=xfb, in1=dx1A,
                                        op=OP.mult)
                nc.vector.tensor_tensor(out=IA1v, in0=IA1v, in1=v1A,
                                        op=OP.add)
                nc.vector.tensor_tensor(out=DYAv, in0=IA1v, in1=IA0v,
                                        op=OP.subtract)
                nc.vector.tensor_tensor(out=VALv, in0=yfb, in1=DYAv,
                                        op=OP.mult)
                nc.vector.tensor_tensor(out=VALv, in0=VALv, in1=IA0v,
                                        op=OP.add)

                def gview(T, g):  # [128, 64] view of grid g in (k,f,h) tile
                    base = 2 * NF * (g // 2) + (g % 2)
                    end = 2 * NF * (g // 2) + 2 * NF
                    return T[:, base:end:2]

                zp = gview(VAL, 0)
                kp = gview(VAL, 1)
                cp = gview(VAL, 2)
                fp = gview(VAL, 3)

                # normals from z-grid diffs
                dx0z = TTQ[:, 0:1, :, 1:2]
                dx1z = TTQ[:, 0:1, :, 3:4]
                dzx = fresh("dzx", (128, NF))
                dzxv = dzx[:, :].rearrange("p (a f b) -> p a f b", a=1, b=1)
                nc.vector.tensor_tensor(out=dzxv, in0=dx1z, in1=dx0z,
                                        op=OP.subtract)
                nc.vector.tensor_tensor(out=dzx[:, :], in0=yf[:, :],
                                        in1=dzx[:, :], op=OP.mult)
                nc.vector.tensor_tensor(out=dzxv, in0=dzxv, in1=dx0z,
                                        op=OP.add)
                q = dzx[:, :]
                p2 = gview(DYA, 0)
                sq1 = fresh("sq1")
                sq2 = fresh("sq2")
                nc.scalar.square(sq1[:, :], q)
                nc.scalar.square(sq2[:, :], p2)
                ss = fresh("ss")
                nc.vector.tensor_tensor(out=ss[:, :], in0=sq1[:, :],
                                        in1=sq2[:, :], op=OP.add)
                nc.vector.tensor_scalar(out=ss[:, :], in0=ss[:, :],
                                        scalar1=100.0, scalar2=1.0,
                                        op0=OP.mult, op1=OP.add)
                rt = fresh("rt")
                nc.scalar.sqrt(rt[:, :], ss[:, :])
                rn = fresh("rn")
                nc.vector.reciprocal(out=rn[:, :], in_=rt[:, :])
                N3 = fresh("N3", (128, 3 * NF))
                nx = N3[:, 0:NF]
                ny = N3[:, NF:2 * NF]
                nz = N3[:, 2 * NF:3 * NF]
                nc.vector.tensor_scalar(out=nx, in0=q, scalar1=-10.0,
                                        scalar2=None, op0=OP.mult)
                nc.vector.tensor_tensor(out=nx, in0=nx, in1=rn[:, :],
                                        op=OP.mult)
                nc.vector.tensor_scalar(out=ny, in0=p2, scalar1=-10.0,
                                        scalar2=None, op0=OP.mult)
                nc.vector.tensor_tensor(out=ny, in0=ny, in1=rn[:, :],
                                        op=OP.mult)
                nc.scalar.copy(nz, rn[:, :])

                def b3(ap2d):  # [128, 64] -> broadcast over 3 comp blocks
                    return ap2d.rearrange("p (a f) -> p a f",
                                          a=1).broadcast_to((128, 3, NF))

                V3c = cur["V3"]
                V3v = V3c[:, :].rearrange("p (c f) -> p c f", c=3)
                N3v = N3[:, :].rearrange("p (c f) -> p c f", c=3)
                dh = fresh("dh")
                nc.vector.tensor_tensor(out=dh[:, :], in0=cur["Zp"][:, :],
                                        in1=zp, op=OP.subtract)
                # xdn = sum_c V3*N3
                M3 = fresh("M3", (128, 3 * NF))
                nc.vector.tensor_tensor(out=M3[:, :], in0=V3c[:, :],
                                        in1=N3[:, :], op=OP.mult)
                xdn = fresh("xdn")
                nc.vector.tensor_reduce(
                    out=xdn[:, :],
                    in_=M3[:, :].rearrange("p (c f) -> p f c", c=3),
                    axis=mybir.AxisListType.X, op=OP.add)

                t1 = fresh("t1")
                t2 = fresh("t2")
                nc.vector.tensor_tensor(out=t1[:, :], in0=kp, in1=dh[:, :],
                                        op=OP.mult)
                nc.vector.tensor_tensor(out=t2[:, :], in0=cp, in1=xdn[:, :],
                                        op=OP.mult)
                nc.vector.tensor_tensor(out=t1[:, :], in0=t1[:, :],
                                        in1=t2[:, :], op=OP.add)
                ctc = fresh("ctc")
                nc.vector.scalar_tensor_tensor(out=ctc[:, :], in0=dh[:, :],
                                               scalar=0.0, in1=NEG[:, :],
                                               op0=OP.is_le, op1=OP.mult)
                Ssc = fresh("Ssc")
                nc.vector.tensor_tensor(out=Ssc[:, :], in0=t1[:, :],
                                        in1=ctc[:, :], op=OP.mult)
                FS3 = fresh("FS3", (128, 3 * NF))
                FS3v = FS3[:, :].rearrange("p (c f) -> p c f", c=3)
                nc.vector.tensor_tensor(out=FS3v, in0=b3(Ssc[:, :]),
                                        in1=N3v, op=OP.mult)
                Nm = fresh("Nm")
                nc.scalar.activation(Nm[:, :], Ssc[:, :], AF.Abs)
                fN = fresh("fN")
                nc.vector.tensor_tensor(out=fN[:, :], in0=fp, in1=Nm[:, :],
                                        op=OP.mult)

                # thrust projection shared across tracks:
                # dv_tau^tk = s_tk * P + Q,  P = td - (td.n) n, Q = xdn*n - V
                tn = fresh("tn")
                nc.vector.tensor_scalar(out=tn[:, :], in0=nx,
                                        scalar1=svc[0][0], scalar2=None,
                                        op0=OP.mult)
                nc.vector.scalar_tensor_tensor(out=tn[:, :], in0=ny,
                                               scalar=svc[0][1],
                                               in1=tn[:, :],
                                               op0=OP.mult, op1=OP.add)
                nc.vector.scalar_tensor_tensor(out=tn[:, :], in0=nz,
                                               scalar=svc[0][2],
                                               in1=tn[:, :],
                                               op0=OP.mult, op1=OP.add)
                TD3 = fresh("TD3", (128, 3 * NF))
                for c in range(3):
                    nc.vector.tensor_scalar(out=TD3[:, c * NF:(c + 1) * NF],
                                            in0=ONES[:, :],
                                            scalar1=svc[0][c], scalar2=None,
                                            op0=OP.mult)
                P3 = fresh("P3", (128, 3 * NF))
                P3v = P3[:, :].rearrange("p (c f) -> p c f", c=3)
                nc.vector.tensor_tensor(out=P3v, in0=b3(tn[:, :]), in1=N3v,
                                        op=OP.mult)
                nc.vector.tensor_tensor(out=P3[:, :], in0=TD3[:, :],
                                        in1=P3[:, :], op=OP.subtract)
                Q3 = fresh("Q3", (128, 3 * NF))
                Q3v = Q3[:, :].rearrange("p (c f) -> p c f", c=3)
                nc.vector.tensor_tensor(out=Q3v, in0=b3(xdn[:, :]), in1=N3v,
                                        op=OP.mult)
                nc.vector.tensor_tensor(out=Q3[:, :], in0=Q3[:, :],
                                        in1=V3c[:, :], op=OP.subtract)

                FF3 = fresh("FF3", (128, 3 * NF))
                for tk in range(2):
                    dvt = fresh(f"dvt{tk}", (128, 3 * NF))
                    nc.vector.scalar_tensor_tensor(out=dvt[:, :],
                                                   in0=P3[:, :],
                                                   scalar=svc[1][tk],
                                                   in1=Q3[:, :],
                                                   op0=OP.mult, op1=OP.add)
                    th3 = fresh(f"th3{tk}", (128, 3 * NF))
                    nc.scalar.activation(th3[:, :], dvt[:, :], AF.Tanh)
                    fNt = fresh(f"fNt{tk}")
                    nc.vector.tensor_tensor(out=fNt[:, :], in0=fN[:, :],
                                            in1=MK[:, tk * NF:(tk + 1) * NF],
                                            op=OP.mult)
                    th3v = th3[:, :].rearrange("p (c f) -> p c f", c=3)
                    if tk == 0:
                        FF3v = FF3[:, :].rearrange("p (c f) -> p c f", c=3)
                        nc.vector.tensor_tensor(out=FF3v, in0=th3v,
                                                in1=b3(fNt[:, :]),
                                                op=OP.mult)
                    else:
                        g3 = fresh("g3", (128, 3 * NF))
                        g3v = g3[:, :].rearrange("p (c f) -> p c f", c=3)
                        nc.vector.tensor_tensor(out=g3v, in0=th3v,
                                                in1=b3(fNt[:, :]),
                                                op=OP.mult)
                        nc.vector.tensor_tensor(out=FF3[:, :],
                                                in0=FF3[:, :],
                                                in1=g3[:, :], op=OP.add)

                P6 = fresh("P6", (128, 8))
                F3 = fresh("F3", (128, 3 * NF))
                for c in range(3):
                    nc.vector.scalar_tensor_tensor(
                        out=F3[:, c * NF:(c + 1) * NF],
                        in0=FS3[:, c * NF:(c + 1) * NF], scalar=0.0,
                        in1=FF3[:, c * NF:(c + 1) * NF],
                        op0=OP.bypass, op1=OP.add,
                        accum_out=P6[:, c:c + 1])
                Fx = F3[:, 0:NF]
                Fy = F3[:, NF:2 * NF]
                Fz = F3[:, 2 * NF:3 * NF]
                tta = fresh("tta"); ttb = fresh("ttb")
                ttc = fresh("ttc"); ttd = fresh("ttd")
                tte = fresh("tte"); ttf = fresh("ttf")
                nc.vector.tensor_tensor(out=tta[:, :], in0=RY[:, :],
                                        in1=Fz, op=OP.mult)
                nc.vector.tensor_tensor(out=ttb[:, :], in0=RZ[:, :],
                                        in1=Fy, op=OP.mult)
                tx = fresh("tx")
                nc.vector.scalar_tensor_tensor(out=tx[:, :], in0=tta[:, :],
                                               scalar=0.0, in1=ttb[:, :],
                                               op0=OP.bypass, op1=OP.subtract,
                                               accum_out=P6[:, 3:4])
                nc.vector.tensor_tensor(out=ttc[:, :], in0=RZ[:, :],
                                        in1=Fx, op=OP.mult)
                nc.vector.tensor_tensor(out=ttd[:, :], in0=RX[:, :],
                                        in1=Fz, op=OP.mult)
                ty = fresh("ty")
                nc.vector.scalar_tensor_tensor(out=ty[:, :], in0=ttc[:, :],
                                               scalar=0.0, in1=ttd[:, :],
                                               op0=OP.bypass, op1=OP.subtract,
                                               accum_out=P6[:, 4:5])
                nc.vector.tensor_tensor(out=tte[:, :], in0=RX[:, :],
                                        in1=Fy, op=OP.mult)
                nc.vector.tensor_tensor(out=ttf[:, :], in0=RY[:, :],
                                        in1=Fx, op=OP.mult)
                tz = fresh("tz")
                nc.vector.scalar_tensor_tensor(out=tz[:, :], in0=tte[:, :],
                                               scalar=0.0, in1=ttf[:, :],
                                               op0=OP.bypass, op1=OP.subtract,
                                               accum_out=P6[:, 5:6])

                # ---- R phase
                PS1 = pspool.tile([8, 8], F32, tag="PS1", name="PS1")
                nc.tensor.matmul(PS1[:, 0:6], L1[:, :], P6[:, 0:6],
                                 start=True, stop=True)
                BCn = fresh("BCn", (8, 16))
                omd = fresh("omd", (8, 3))
                nc.vector.tensor_tensor(out=omd[:, :], in0=PS1[:, 3:6],
                                        in1=SCT[:, 0:3], op=OP.mult)
                nc.vector.tensor_scalar(out=omd[:, :], in0=omd[:, :],
                                        scalar1=OMEGA_MAX, scalar2=-OMEGA_MAX,
                                        op0=OP.min, op1=OP.max)
                xdd = fresh("xdd", (8, 3))
                nc.vector.scalar_tensor_tensor(out=xdd[:, :], in0=PS1[:, 0:3],
                                               scalar=1.0 / MASS,
                                               in1=SCT[:, 3:6],
                                               op0=OP.mult, op1=OP.add)
                BCp = cur["BC"]
                nc.vector.scalar_tensor_tensor(out=BCn[:, 3:6],
                                               in0=xdd[:, :], scalar=C_RK4,
                                               in1=BCp[:, 3:6],
                                               op0=OP.mult, op1=OP.add)
                nc.vector.scalar_tensor_tensor(out=BCn[:, 0:3],
                                               in0=BCn[:, 3:6], scalar=C_RK4,
                                               in1=BCp[:, 0:3],
                                               op0=OP.mult, op1=OP.add)
                nc.vector.scalar_tensor_tensor(out=BCn[:, 6:9],
                                               in0=omd[:, :], scalar=C_RK4,
                                               in1=BCp[:, 6:9],
                                               op0=OP.mult, op1=OP.add)
                nc.scalar.copy(XH[:, 3 * t:3 * t + 3], BCn[:, 0:3])

                # small-angle exact-in-f32 Taylor: s2 = (th*DT)^2 <= 0.015
                # a = DT*(1 + s2*(s2/120 - 1/6)); b = DT^2*(0.5 + s2*(s2/720 - 1/24))
                th2 = fresh("th2", (8, 1))
                sqw = fresh("sqw", (8, 3))
                nc.scalar.activation(sqw[:, :], BCn[:, 6:9], AF.Square,
                                     accum_out=th2[:, :])
                s2 = fresh("s2", (8, 1))
                nc.vector.tensor_scalar(out=s2[:, :], in0=th2[:, :],
                                        scalar1=DT * DT, scalar2=None,
                                        op0=OP.mult)
                ua = fresh("ua", (8, 1))
                nc.vector.tensor_scalar(out=ua[:, :], in0=s2[:, :],
                                        scalar1=1.0 / 120.0,
                                        scalar2=-1.0 / 6.0,
                                        op0=OP.mult, op1=OP.add)
                av = fresh("av", (8, 1))
                nc.vector.scalar_tensor_tensor(out=av[:, :], in0=ua[:, :],
                                               scalar=s2[:, :],
                                               in1=SCT[:, 6:7],
                                               op0=OP.mult, op1=OP.add)
                nc.vector.tensor_scalar(out=av[:, :], in0=av[:, :],
                                        scalar1=DT, scalar2=None, op0=OP.mult)
                ub = fresh("ub", (8, 1))
                nc.vector.tensor_scalar(out=ub[:, :], in0=s2[:, :],
                                        scalar1=1.0 / 720.0,
                                        scalar2=-1.0 / 24.0,
                                        op0=OP.mult, op1=OP.add)
                bv = fresh("bv", (8, 1))
                nc.vector.scalar_tensor_tensor(out=bv[:, :], in0=ub[:, :],
                                               scalar=s2[:, :],
                                               in1=SCT[:, 7:8],
                                               op0=OP.mult, op1=OP.add)
                nc.vector.tensor_scalar(out=bv[:, :], in0=bv[:, :],
                                        scalar1=DT * DT, scalar2=None,
                                        op0=OP.mult)
                MM = fresh("MM", (8, 9))
                u = [BCn[:, 6 + c:7 + c] for c in range(3)]
                dd = fresh("dd", (8, 3))
                nc.vector.tensor_tensor(out=dd[:, :], in0=BCn[:, 6:9],
                                        in1=BCn[:, 6:9], op=OP.mult)
                nc.vector.tensor_scalar(out=dd[:, :], in0=dd[:, :],
                                        scalar1=th2[:, :], scalar2=None,
                                        op0=OP.subtract)
                nc.vector.tensor_scalar(out=dd[:, :], in0=dd[:, :],
                                        scalar1=bv[:, :], scalar2=None,
                                        op0=OP.mult)
                nc.vector.tensor_scalar(out=MM[:, 0:9:4], in0=dd[:, :],
                                        scalar1=1.0, scalar2=None, op0=OP.add)
                for (i, j, kk, sgn) in ((0, 1, 2, +1), (0, 2, 1, -1),
                                        (1, 2, 0, +1)):
                    hp = fresh(f"hp{i}{j}", (8, 1))
                    hq = fresh(f"hq{i}{j}", (8, 1))
                    nc.vector.tensor_tensor(out=hp[:, :], in0=u[i], in1=u[j],
                                            op=OP.mult)
                    nc.vector.tensor_scalar(out=hp[:, :], in0=hp[:, :],
                                            scalar1=bv[:, :], scalar2=None,
                                            op0=OP.mult)
                    nc.vector.tensor_tensor(out=hq[:, :], in0=u[kk],
                                            in1=av[:, :], op=OP.mult)
                    a_ij = MM[:, 3 * i + j:3 * i + j + 1]
                    a_ji = MM[:, 3 * j + i:3 * j + i + 1]
                    if sgn > 0:
                        nc.vector.tensor_tensor(out=a_ij, in0=hp[:, :],
                                                in1=hq[:, :], op=OP.subtract)
                        nc.vector.tensor_tensor(out=a_ji, in0=hp[:, :],
                                                in1=hq[:, :], op=OP.add)
                    else:
                        nc.vector.tensor_tensor(out=a_ij, in0=hp[:, :],
                                                in1=hq[:, :], op=OP.add)
                        nc.vector.tensor_tensor(out=a_ji, in0=hp[:, :],
                                                in1=hq[:, :], op=OP.subtract)
                Rn = fresh("Rn", (8, 9))
                for b in range(3):
                    nc.vector.tensor_scalar(out=Rn[:, b:9:3],
                                            in0=RB[:, 0:9:3],
                                            scalar1=MM[:, b:b + 1],
                                            scalar2=None, op0=OP.mult)
                    nc.vector.scalar_tensor_tensor(out=Rn[:, b:9:3],
                                                   in0=RB[:, 1:9:3],
                                                   scalar=MM[:, 3 + b:4 + b],
                                                   in1=Rn[:, b:9:3],
                                                   op0=OP.mult, op1=OP.add)
                    nc.vector.scalar_tensor_tensor(out=Rn[:, b:9:3],
                                                   in0=RB[:, 2:9:3],
                                                   scalar=MM[:, 6 + b:7 + b],
                                                   in1=Rn[:, b:9:3],
                                                   op0=OP.mult, op1=OP.add)
                nc.vector.tensor_copy(out=RB[:, 0:9], in_=Rn[:, 0:9])
                if t < nsteps - 1:
                    ssr = fresh("ssr", (8, 1))
                    sqr = fresh("sqr", (8, 3))
                    nc.scalar.activation(sqr[:, :], Rn[:, 0:9:3], AF.Square,
                                         accum_out=ssr[:, :])
                    rsr = fresh("rsr", (8, 1))
                    nc.vector.tensor_scalar(out=rsr[:, :], in0=ssr[:, :],
                                            scalar1=-0.5, scalar2=1.5,
                                            op0=OP.mult, op1=OP.add)
                    nc.vector.tensor_scalar(out=BCn[:, 9:12],
                                            in0=Rn[:, 0:9:3],
                                            scalar1=rsr[:, :], scalar2=None,
                                            op0=OP.mult)
                    nc.vector.tensor_copy(out=BCn[:, 12:14],
                                          in_=TV[:, 2 * t + 2:2 * t + 4])
                    nc.vector.memset(BCn[:, 14:16], 0.0)
                    PS2n = pspool.tile([128, 16], F32, tag="PS2", name="PS2n")
                    nc.tensor.matmul(PS2n[:, :], L2[:, :], BCn[:, :],
                                     start=True, stop=True)
                else:
                    PS2n = None

                cur = dict(Xp=Xn, Yp=Yn, Zp=Zn, V3=V3n,
                           xf=xf_n, yf=yf_n, GA=GA_n, PS2=PS2n,
                           BC=BCn)

            nc.sync.dma_start(out=out_d[:, :], in_=XH[:, :])

    nc.compile()
    return nc


def prep_core_inputs(z_grid, stiffness, damping, friction, controls,
                     x_points0, track_ids, core):
    r0 = core * 8
    import ml_dtypes
    grids = [z_grid, stiffness, damping, friction]
    wins = np.zeros((128, 8 * NE), ml_dtypes.bfloat16)
    for r in range(8):
        for lane in range(2):
            octw = np.empty((WN, WN, 8), np.float32)
            for h in range(2):
                G = np.asarray(grids[2 * lane + h][r0 + r], np.float32)
                sub = G[WX0:WX0 + WN + 1, WY0:WY0 + WN + 1].astype(
                    ml_dtypes.bfloat16)
                subf = sub.astype(np.float32)
                octw[:, :, 4 * h + 0] = subf[0:WN, 0:WN]
                octw[:, :, 4 * h + 1] = subf[1:WN + 1, 0:WN] - subf[0:WN, 0:WN]
                octw[:, :, 4 * h + 2] = subf[0:WN, 1:WN + 1]
                octw[:, :, 4 * h + 3] = subf[1:WN + 1, 1:WN + 1] - \
                    subf[0:WN, 1:WN + 1]
            wins[16 * r + lane, :] = octw.reshape(-1).astype(ml_dtypes.bfloat16)
    pts = np.zeros((128, 3 * NF), np.float32)
    for r in range(8):
        P = np.asarray(x_points0[r0 + r], np.float32)
        for c in range(3):
            pts[16 * r:16 * r + 16, c * NF:(c + 1) * NF] = \
                P[:, c].reshape(NF, 16).T
    msk = np.zeros((128, 2 * NF), np.float32)
    tid = np.asarray(track_ids)
    for tk in range(2):
        blk = (tid == tk).astype(np.float32).reshape(NF, 16).T
        for r in range(8):
            msk[16 * r:16 * r + 16, tk * NF:(tk + 1) * NF] = blk
    tv = np.zeros((8, 2 * T_STEPS), np.float32)
    ctl = np.asarray(controls, np.float32)
    v = ctl[:, r0:r0 + 8, 0]
    w = ctl[:, r0:r0 + 8, 1]
    tv[:, 0::2] = (v - w * ROBOT_LY / 2.0).T
    tv[:, 1::2] = (v + w * ROBOT_LY / 2.0).T
    l1 = np.zeros((128, 8), np.float32)
    for p in range(128):
        l1[p, p // 16] = 1.0
    l2 = np.ascontiguousarray(l1.T)
    sc = np.zeros((8, 8), np.float32)
    sc[:, 0:3] = [1.0, 1.0 / 3.5, 1.0 / 4.0]
    sc[:, 5] = -GRAV
    sc[:, 6] = 1.0
    sc[:, 7] = 0.5
    return dict(wins=wins, pts=pts, msk=msk, tv=tv, l1=l1, l2=l2, sc=sc)


def postprocess(results):
    out = np.zeros((T_STEPS, 64, 3), np.float32)
    for core in range(8):
        o = np.asarray(results[core]["out"])
        for r in range(8):
            out[:, core * 8 + r, :] = o[r].reshape(T_STEPS, 3)
    return out


# ----------------------------------------------------------------------------
# Harness entry point: full inputs in, full output out.
# ----------------------------------------------------------------------------
_NC_CACHE = {}


def kernel(z_grid, stiffness, damping, friction, controls, x_points0,
           track_ids):
    import numpy as np
    from concourse.bass_utils import run_bass_kernel_spmd

    z_grid = np.asarray(z_grid, np.float32)
    stiffness = np.asarray(stiffness, np.float32)
    damping = np.asarray(damping, np.float32)
    friction = np.asarray(friction, np.float32)
    controls = np.asarray(controls, np.float32)
    x_points0 = np.asarray(x_points0, np.float32)
    track_ids = np.asarray(track_ids, np.int32)

    if "nc" not in _NC_CACHE:
        _NC_CACHE["nc"] = build(nsteps=T_STEPS)
    nc = _NC_CACHE["nc"]

    in_maps = [prep_core_inputs(z_grid, stiffness, damping, friction,
                                controls, x_points0, track_ids, core)
               for core in range(8)]
    res = run_bass_kernel_spmd(nc, in_maps, core_ids=list(range(8)))
    return postprocess(res.results)

